# revision 3
# baseline (speedup 1.0000x reference)
"""Multi-head attention block (B=32,S=512,D=768,H=12) on 8 TRN2 NeuronCores.

Sharding: data-parallel over batch (4 batches/core), weights replicated,
no collectives. Host pre-transposes x and the weight matrices so the
device kernel is a pure matmul pipeline (no on-chip transposes):

  per core (4 batches), all matmul operands bf16 (host-converted), fp32
  accumulation in PSUM:
    yT[o,t]  = Wqkv xT for q,k rows (o on partitions -> ACT per-partition
             bias during the psum->sbuf copy)
    v[t,o]   natural, bias added from a partition-broadcast tile during the
             interleave copy; stored with an all-ones column per head:
             [v_h | 1] is the stationary operand of the av matmul, so row
             64 of the av output is the softmax denominator for free.
    per head: scoresT[s,t] = kT^T qT (K=64), exp on ACT ([128,1024] ops,
             scale folded, Exp table stays resident), av+sums in one
             matmul.  Normalization runs in waves of 3 heads: sums rows
             stack via DMA as 4x128 blocks at partition offsets 0/32/64,
             one DVE reciprocal per wave at free-dim 128, DMA hop to
             partition 0, gpsimd partition-broadcast, DVE bf16 multiply
             (odd heads partition-shifted into the packed avT via DMA).
    out[t,:] = avT^T WpT + combo (K=1 ones matmul); ACT-free final copy
             adds combo via DVE tensor-tensor.

Schedule: software-pipelined qkv(b) -> proj(b-1) -> attn(b), with x
prefetch and one yT chunk-pair of batch b+1 interleaved into the tail of
attn(b).  Measured ~280 us on 8 cores (rel err ~6e-3 vs fp32 reference).
"""

import sys

if "/opt/trn_rl_repo" not in sys.path:
    sys.path.insert(0, "/opt/trn_rl_repo")

from contextlib import ExitStack

import numpy as np

import concourse.tile as tile
from concourse import bacc, mybir
from concourse.bass_utils import run_bass_kernel_spmd

B, S, D = 32, 512, 768
H, HD = 12, 64
SCALE = HD**-0.5
NCORES = 8
NB = B // NCORES  # batches per core
P = 128
TCH = S // P  # token chunks per batch
DCH = D // P  # d chunks
QKC = 2 * D // P  # o-chunks holding q,k
NHALF = D // 2  # 384: N-tile for v/proj matmuls
F32 = mybir.dt.float32
F32R = mybir.dt.float32r
BF16 = mybir.dt.bfloat16
EXP = mybir.ActivationFunctionType.Exp


def _act_reciprocal(nc, out_ap, in_ap):
    """Raw ACT-table reciprocal (~1e-3 rel for |x| >= ~2.5; softmax sums here
    are >= ~50). The bass wrapper refuses Reciprocal for general use; emit
    InstActivation directly."""
    eng = nc.scalar
    ins_ = [eng.lower_ap(in_ap)]
    for arg in (0.0, 1.0, 0.0):  # bias, scale, alpha
        ins_.append(mybir.ImmediateValue(dtype=F32, value=arg))
    return eng.add_instruction(
        mybir.InstActivation(
            name=eng.bass.get_next_instruction_name(),
            func=mybir.ActivationFunctionType.Reciprocal,
            ins=ins_,
            outs=[eng.lower_ap(out_ap)],
        )
    )


def build_nc():
    nc = bacc.Bacc(None, target_bir_lowering=False, debug=False)
    xT = nc.declare_dram_parameter("xT", [NB, D, S], BF16, isOutput=False)
    wqkvT = nc.declare_dram_parameter("wqkvT", [D, 3 * D], BF16, isOutput=False)
    wpT = nc.declare_dram_parameter("wpT", [D, D], BF16, isOutput=False)
    bqkv = nc.declare_dram_parameter("bqkv", [3 * D], F32, isOutput=False)
    combo = nc.declare_dram_parameter("combo", [D], BF16, isOutput=False)
    bv16 = nc.declare_dram_parameter("bv16", [D], BF16, isOutput=False)
    out = nc.declare_dram_parameter("out", [NB, S, D], F32, isOutput=True)

    WAVE = 6

    with ExitStack() as ctx:
        tc = ctx.enter_context(tile.TileContext(nc))
        wp = ctx.enter_context(tc.tile_pool(name="weights", bufs=1))
        sb = ctx.enter_context(tc.tile_pool(name="work", bufs=1))
        ps = ctx.enter_context(tc.tile_pool(name="psum", bufs=1, space="PSUM"))

        # ---- persistent weights / constants (q,k columns first) ----
        wq_t = [
            wp.tile([P, 3 * D], BF16, name=f"wqkvT{d}", tag=f"wqkvT{d}")
            for d in range(DCH)
        ]
        for d in range(DCH):
            eng = nc.sync if d % 2 == 0 else nc.scalar
            eng.dma_start(
                out=wq_t[d][:, : 2 * D], in_=wqkvT[d * P : (d + 1) * P, : 2 * D]
            )
        bcols = []
        for c in range(QKC):
            t = wp.tile([P, 1], F32, name=f"bcol{c}", tag=f"bcol{c}")
            nc.sync.dma_start(
                out=t, in_=bqkv[c * P : (c + 1) * P].rearrange("(p o) -> p o", o=1)
            )
            bcols.append(t)
        bvrow = wp.tile([1, D], BF16, name="bvrow", tag="bvrow")
        nc.sync.dma_start(out=bvrow, in_=bv16.rearrange("(o f) -> o f", o=1))
        bvb = wp.tile([P, D], BF16, name="bvb", tag="bvb")
        nc.gpsimd.partition_broadcast(bvb, bvrow)
        ones = wp.tile([1, P], BF16, name="ones", tag="ones")
        nc.vector.memset(ones, 1.0)
        for d in range(DCH):
            eng = nc.sync if d % 2 == 0 else nc.scalar
            eng.dma_start(
                out=wq_t[d][:, 2 * D :], in_=wqkvT[d * P : (d + 1) * P, 2 * D :]
            )
        wp_t = []
        for d in range(DCH):
            t = wp.tile([P, D], BF16, name=f"wpT{d}", tag=f"wpT{d}")
            nc.sync.dma_start(out=t, in_=wpT[d * P : (d + 1) * P, :])
            wp_t.append(t)
        comborow = wp.tile([1, D], BF16, name="comborow", tag="comborow")
        nc.sync.dma_start(out=comborow, in_=combo.rearrange("(o f) -> o f", o=1))
        cbb = wp.tile([P, D], BF16, name="cbb", tag="cbb")
        nc.gpsimd.partition_broadcast(cbb, comborow)

        def emit_x_load(b):
            xt = []
            for d in range(DCH):
                t = sb.tile([P, S], BF16, name=f"xT_b{b}_{d}", tag=f"xT{d}", bufs=2)
                nc.gpsimd.dma_start(out=t, in_=xT[b, d * P : (d + 1) * P, :])
                xt.append(t)
            return xt

        def emit_yT_chunk(b, xt, c):
            pt = ps.tile([P, S], F32, name=f"yTps_b{b}_{c}", tag="mm", bufs=2)
            for d in range(DCH):
                nc.tensor.matmul(
                    out=pt,
                    lhsT=wq_t[d][:, c * P : (c + 1) * P],
                    rhs=xt[d],
                    start=(d == 0),
                    stop=(d == DCH - 1),
                )
            st = sb.tile([P, S], BF16, name=f"yT_b{b}_{c}", tag=f"yT{c}", bufs=2)
            nc.scalar.activation(
                st, pt, mybir.ActivationFunctionType.Identity, bias=bcols[c]
            )
            return st

        def emit_v_tile(b, xt, ti):
            vtile = sb.tile(
                [P, H * (HD + 1)], BF16, name=f"v_b{b}_{ti}", tag=f"v{ti}", bufs=2
            )
            nc.vector.memset(
                vtile.rearrange("p (h k) -> p h k", k=HD + 1)[:, :, HD : HD + 1],
                1.0,
            )
            for half in range(2):
                pv = ps.tile(
                    [P, NHALF], F32, name=f"vps_b{b}_{ti}_{half}", tag="mm", bufs=2
                )
                o0 = 2 * D + half * NHALF
                for d in range(DCH):
                    nc.tensor.matmul(
                        out=pv,
                        lhsT=xt[d][:, ti * P : (ti + 1) * P],
                        rhs=wq_t[d][:, o0 : o0 + NHALF],
                        start=(d == 0),
                        stop=(d == DCH - 1),
                    )
                nc.vector.tensor_tensor(
                    out=vtile.rearrange("p (h k) -> p h k", k=HD + 1)[
                        :, 6 * half : 6 * (half + 1), 0:HD
                    ],
                    in0=pv.rearrange("p (h k) -> p h k", k=HD),
                    in1=bvb[:, half * NHALF : (half + 1) * NHALF].rearrange(
                        "p (h k) -> p h k", k=HD
                    ),
                    op=mybir.AluOpType.add,
                )
            return vtile

        def emit_scores(b, h, yt):
            hp = (h % 2) * HD
            qs = yt[h // 2][hp : hp + HD, :]
            ks = yt[6 + h // 2][hp : hp + HD, :]
            exps = []
            for jp in range(2):
                pt = ps.tile(
                    [P, 2 * S], F32, name=f"sc_b{b}_h{h}_{jp}", tag="sc", bufs=3
                )
                for jj in range(2):
                    j = 2 * jp + jj
                    nc.tensor.matmul(
                        out=pt[:, jj * S : (jj + 1) * S],
                        lhsT=ks[:, j * P : (j + 1) * P],
                        rhs=qs,
                        start=True,
                        stop=True,
                    )
                et = sb.tile(
                    [P, 2 * S], BF16, name=f"expT_b{b}_h{h}_{jp}", tag="expT",
                    bufs=4,
                )
                nc.scalar.activation(et, pt, EXP, scale=SCALE)
                exps.append(et)
            return exps

        def emit_av(b, h, exps, vt, avt, state):
            pav = ps.tile([HD + 1, S], F32, name=f"av_b{b}_h{h}", tag="mm", bufs=2)
            for j in range(TCH):
                nc.tensor.matmul(
                    out=pav,
                    lhsT=vt[j][:, h * (HD + 1) : (h + 1) * (HD + 1)],
                    rhs=exps[j // 2][:, (j % 2) * S : (j % 2 + 1) * S],
                    start=(j == 0),
                    stop=(j == TCH - 1),
                )
            # DVE copy frees the psum bank fast and gives the sums row a
            # DMA-able SBUF home; bf16 out enables 2x DVE for the mults.
            avsb = sb.tile([HD + 1, S], BF16, name=f"avsb_b{b}_h{h}", tag="avsb",
                           bufs=8)
            nc.vector.tensor_copy(avsb, pav)
            state["avsbs"].append(avsb)
            wi = h - state["wave_start"]
            if state["fastrec"]:
                nc.sync.dma_start(
                    out=state["stacked"][32 * wi : 32 * wi + 4, :],
                    in_=avsb[HD : HD + 1, :],
                )
            else:
                nc.sync.dma_start(
                    out=state["stacked"][wi : wi + 1, :],
                    in_=avsb[HD : HD + 1, :],
                )
            if h in state["wave_ends"]:
                w0 = state["wave_start"]
                nw = h - w0 + 1
                if state["fastrec"]:
                    recw = sb.tile([P, P], F32, name=f"recw_b{b}_h{h}",
                                   tag="recw", bufs=2)
                    nc.vector.reciprocal(recw, state["stacked"])
                else:
                    recw = sb.tile([WAVE, S], F32, name=f"recw_b{b}_h{h}",
                                   tag="recw", bufs=2)
                    nc.vector.reciprocal(recw[:nw, :], state["stacked"][:nw, :])
                for hh in range(w0, h + 1):
                    wj = hh - w0
                    rrow = sb.tile([1, S], BF16, name=f"rrow_b{b}_h{hh}",
                                   tag="rrow", bufs=2 * WAVE)
                    if state["fastrec"]:
                        nc.gpsimd.dma_start(
                            out=rrow, in_=recw[32 * wj : 32 * wj + 4, :]
                        )
                    else:
                        nc.gpsimd.dma_start(out=rrow, in_=recw[wj : wj + 1, :])
                    bc = sb.tile([HD, S], BF16, name=f"bc_b{b}_h{hh}", tag="bc",
                                 bufs=WAVE + 1)
                    nc.gpsimd.partition_broadcast(bc, rrow)
                    c = hh // 2
                    src_av = state["avsbs"][hh]
                    if hh % 2 == 0:
                        nc.vector.tensor_mul(avt[c][:HD, :], src_av[:HD, :], bc)
                    else:
                        tmp = sb.tile([HD, S], BF16, name=f"avtmp_b{b}_h{hh}",
                                      tag="avtmp", bufs=4)
                        nc.vector.tensor_mul(tmp, src_av[:HD, :], bc)
                        nc.sync.dma_start(out=avt[c][HD : 2 * HD, :], in_=tmp)
                if state["fastrec"]:
                    state["stacked"] = sb.tile(
                        [P, P], BF16, name=f"stk_b{b}_h{h}", tag="stacked", bufs=2
                    )
                    nc.vector.memset(state["stacked"], 1.0)
                else:
                    state["stacked"] = sb.tile(
                        [WAVE, S], BF16, name=f"stk_b{b}_h{h}", tag="stacked",
                        bufs=2,
                    )
                state["wave_start"] = h + 1

        def emit_proj(b, avt):
            # two waves of 4 interleaved accumulation groups; [bias,d0..d2]
            # first so the PE has ready work while the last attention wave's
            # avt[3..5] normalization drains.
            tags = ["sc", "sc", "mm", "mm"]
            fts = {}
            for wave_t in range(2):
                groups = []
                for k in range(4):
                    ti = 2 * wave_t + (k // 2)
                    half = k % 2
                    pf = ps.tile(
                        [P, NHALF], F32, name=f"fps_b{b}_{ti}_{half}",
                        tag=tags[k], bufs={"mm": 2, "sc": 3}[tags[k]],
                    )
                    groups.append((pf, ti, half))
                for pf, ti, half in groups:
                    for d in range(4):
                        nc.tensor.matmul(
                            out=pf,
                            lhsT=avt[d][:, ti * P : (ti + 1) * P],
                            rhs=wp_t[d][:, half * NHALF : (half + 1) * NHALF],
                            start=(d == 0),
                            stop=False,
                        )
                for pf, ti, half in groups:
                    for d in range(4, DCH):
                        nc.tensor.matmul(
                            out=pf,
                            lhsT=avt[d][:, ti * P : (ti + 1) * P],
                            rhs=wp_t[d][:, half * NHALF : (half + 1) * NHALF],
                            start=False,
                            stop=(d == DCH - 1),
                        )
                    if ti not in fts:
                        fts[ti] = sb.tile(
                            [P, D], F32, name=f"fin_b{b}_{ti}", tag="fin", bufs=3
                        )
                    nc.vector.tensor_tensor(
                        out=fts[ti][:, half * NHALF : (half + 1) * NHALF],
                        in0=pf,
                        in1=cbb[:, half * NHALF : (half + 1) * NHALF],
                        op=mybir.AluOpType.add,
                    )
                    if half == 1:
                        nc.sync.dma_start(
                            out=out[b, ti * P : (ti + 1) * P, :], in_=fts[ti]
                        )

        # ---- main schedule: qkv(b) -> proj(b-1) -> attn(b) ----
        prev = None
        pre = {}
        xt = emit_x_load(0)
        for b in range(NB):
            yt = [None] * QKC
            for hp in range(6):
                for c in (hp, 6 + hp):
                    yt[c] = pre[c] if c in pre else emit_yT_chunk(b, xt, c)
            vt = [
                pre["v0"] if ti == 0 and "v0" in pre else emit_v_tile(b, xt, ti)
                for ti in range(TCH)
            ]
            if prev is not None:
                emit_proj(b - 1, prev)
            if b + 1 < NB:
                xt = emit_x_load(b + 1)
            avt = [
                sb.tile([P, S], BF16, name=f"avT_b{b}_{c}", tag=f"avT{c}", bufs=2)
                for c in range(DCH)
            ]
            fastrec = True
            state = {
                "avsbs": [],
                "wave_start": 0,
                "fastrec": fastrec,
                "wave_ends": {2, 5, 8, 10, 11} if b == NB - 1 else {2, 5, 8, 11},
                "stacked": (
                    sb.tile([P, P], BF16, name=f"stk_b{b}_init", tag="stacked",
                            bufs=2)
                    if fastrec
                    else sb.tile([WAVE, S], BF16, name=f"stk_b{b}_init",
                                 tag="stacked", bufs=2)
                ),
            }
            if fastrec:
                nc.vector.memset(state["stacked"], 1.0)
            pre_next = {}
            prev_exps = None
            for h in range(H):
                cur_exps = emit_scores(b, h, yt)
                if h > 0:
                    emit_av(b, h - 1, prev_exps, vt, avt, state)
                prev_exps = cur_exps
                if h == 10 and b + 1 < NB:
                    pre_next[0] = emit_yT_chunk(b + 1, xt, 0)
                    pre_next[6] = emit_yT_chunk(b + 1, xt, 6)
            emit_av(b, H - 1, prev_exps, vt, avt, state)
            if b + 1 < NB:
                pre_next["v0"] = emit_v_tile(b + 1, xt, 0)
            pre = pre_next
            prev = avt
        emit_proj(NB - 1, prev)

    nc.compile()
    return nc


_CACHE = {}


def _get_nc():
    if "nc" not in _CACHE:
        _CACHE["nc"] = build_nc()
    return _CACHE["nc"]


def _prepare_in_maps(x, qkv_w, qkv_b, proj_w, proj_b):
    x = np.asarray(x, dtype=np.float32)
    qkv_w = np.asarray(qkv_w, dtype=np.float32)
    qkv_b = np.asarray(qkv_b, dtype=np.float32)
    proj_w = np.asarray(proj_w, dtype=np.float32)
    proj_b = np.asarray(proj_b, dtype=np.float32)
    import ml_dtypes

    bf16 = ml_dtypes.bfloat16
    wqkvT = np.ascontiguousarray(qkv_w.T).astype(bf16)
    wpT = np.ascontiguousarray(proj_w.T).astype(bf16)
    combo = proj_b.astype(bf16)  # v-bias flows through softmax via bvrow
    bv16 = qkv_b[2 * D :].astype(bf16)
    in_maps = []
    for c in range(NCORES):
        xs = x[c * NB : (c + 1) * NB]
        xTs = np.ascontiguousarray(xs.transpose(0, 2, 1)).astype(bf16)
        in_maps.append(
            {
                "xT": xTs,
                "wqkvT": wqkvT,
                "wpT": wpT,
                "bqkv": qkv_b,
                "combo": combo,
                "bv16": bv16,
            }
        )
    return in_maps


def kernel(x, qkv_w, qkv_b, proj_w, proj_b):
    nc = _get_nc()
    in_maps = _prepare_in_maps(x, qkv_w, qkv_b, proj_w, proj_b)
    res = run_bass_kernel_spmd(nc, in_maps, core_ids=list(range(NCORES)))
    return np.concatenate([res.results[i]["out"] for i in range(NCORES)], axis=0)

